# revision 9
# baseline (speedup 1.0000x reference)
"""CNN-LSTM-GCN kernel for 8 Trainium2 NeuronCores.

Self-contained: hardcodes shapes from the problem spec.
  x:          [10000, 32, 128] f32
  edge_index: [2, 160000] int
  output:     [10000, 128] f32

Sharding: nodes split 1250/core (padded to 1280), edges partitioned by dst,
weights replicated. One AllGather for the GCN source rows; the graph
aggregation is done with indirect-DMA gathers that accumulate (CCE add).
"""

import numpy as np
import ml_dtypes

import concourse.bass as bass
import concourse.bacc as bacc
import concourse.mybir as mybir
import concourse.tile as tile
from concourse.bass_utils import run_bass_kernel_spmd
from concourse.masks import make_identity

# ---- problem constants ----
N = 10000
T = 32
F = 128
GH = 32          # GCN hidden
CH = 64          # conv channels
LH = 128         # LSTM hidden
NCORES = 8
NL = N // NCORES          # 1250 real nodes per core
NLP = 1280                # padded nodes per core
NB = NLP // 128           # 10 blocks of 128 dst nodes
VAG = NCORES * NLP        # 10240 rows in the all-gathered table
SENT = VAG                # sentinel index -> zero row
D = T * GH                # 1024 features per node row after GCN transform
NT_SL = [(0, 512), (512, 512), (1024, 256)]  # node tiles
TC = T - 1                # conv output timesteps (31)

FP32 = mybir.dt.float32
BF16 = mybir.dt.bfloat16
INT32 = mybir.dt.int32
AF = mybir.ActivationFunctionType
ALU = mybir.AluOpType


def _build_program(pass_counts, nphases=99):
    """Trace the SPMD bass program. pass_counts[b] = gather passes for block b
    (shared by all cores). nphases limits how many phases run (debug)."""
    nc = bacc.Bacc("TRN2", target_bir_lowering=False, debug=False,
                   num_devices=NCORES)

    PASS_TOT = int(sum(pass_counts))

    # ---- I/O ----
    xT_d = nc.dram_tensor("xT", [T, F, NLP], FP32, kind="ExternalInput")
    gidx_d = nc.dram_tensor("gidx", [128, PASS_TOT], INT32, kind="ExternalInput")
    dinv_d = nc.dram_tensor("dinv", [128, NB], FP32, kind="ExternalInput")
    gcnw_d = nc.dram_tensor("gcn_w", [F, GH], FP32, kind="ExternalInput")
    gcnb_d = nc.dram_tensor("gcn_b_rep", [128, 1], FP32, kind="ExternalInput")
    cwp_d = nc.dram_tensor("conv_wp", [5, 128, CH], BF16, kind="ExternalInput")
    cb_d = nc.dram_tensor("conv_b_rep", [128, 1], FP32, kind="ExternalInput")
    wih_d = nc.dram_tensor("w_ihT", [2, 128, 4 * LH], BF16, kind="ExternalInput")
    whh_d = nc.dram_tensor("w_hhT", [LH, 4 * LH], FP32, kind="ExternalInput")
    lb_d = nc.dram_tensor("lstm_b", [128, 4], FP32, kind="ExternalInput")
    f1w_d = nc.dram_tensor("fc1_w", [LH, 64], FP32, kind="ExternalInput")
    f1b_d = nc.dram_tensor("fc1_b", [64, 1], FP32, kind="ExternalInput")
    f2w_d = nc.dram_tensor("fc2_w", [64, F], FP32, kind="ExternalInput")
    f2b_d = nc.dram_tensor("fc2_b", [128, 1], FP32, kind="ExternalInput")
    out_d = nc.dram_tensor("outT", [F, NL], FP32, kind="ExternalOutput")

    cc_in = nc.dram_tensor("cc_in", [NLP, D], FP32)
    cc_out = nc.dram_tensor("cc_out", [VAG + 1, D], FP32, addr_space="Shared")

    with tile.TileContext(nc) as tc:
        with tc.tile_pool(name="wpool", bufs=1) as wp:
            # ---- persistent weights ----
            ident = wp.tile([128, 128], FP32)
            make_identity(nc, ident[:])
            gcnw = wp.tile([F, GH], FP32)
            nc.sync.dma_start(out=gcnw[:], in_=gcnw_d[:])
            gcnb = wp.tile([128, 1], FP32)
            nc.sync.dma_start(out=gcnb[:], in_=gcnb_d[:])
            cwp = wp.tile([128, 5 * CH], BF16)
            for k5 in range(5):
                nc.sync.dma_start(out=cwp[:, k5 * CH:(k5 + 1) * CH],
                                  in_=cwp_d[k5])
            cb = wp.tile([128, 1], FP32)
            nc.sync.dma_start(out=cb[:], in_=cb_d[:])
            wih = wp.tile([128, 2 * 4 * LH], BF16)
            for k2 in range(2):
                nc.sync.dma_start(out=wih[:, k2 * 4 * LH:(k2 + 1) * 4 * LH],
                                  in_=wih_d[k2])
            whh = wp.tile([LH, 4 * LH], FP32)
            nc.sync.dma_start(out=whh[:], in_=whh_d[:])
            lb = wp.tile([128, 4], FP32)
            nc.sync.dma_start(out=lb[:], in_=lb_d[:])
            f1w = wp.tile([LH, 64], FP32)
            nc.sync.dma_start(out=f1w[:], in_=f1w_d[:])
            f1b = wp.tile([64, 1], FP32)
            nc.sync.dma_start(out=f1b[:], in_=f1b_d[:])
            f2w = wp.tile([64, F], FP32)
            nc.sync.dma_start(out=f2w[:], in_=f2w_d[:])
            f2b = wp.tile([128, 1], FP32)
            nc.sync.dma_start(out=f2b[:], in_=f2b_d[:])
            idx_all = wp.tile([128, PASS_TOT], INT32)
            nc.sync.dma_start(out=idx_all[:], in_=gidx_d[:])
            dinv = wp.tile([128, NB], FP32)
            nc.sync.dma_start(out=dinv[:], in_=dinv_d[:])
            zrow = wp.tile([1, D], FP32)
            nc.vector.memset(zrow[:], 0.0)

            # ---- Phase A: h^T = gcn_w^T @ x^T (scaled x), transpose to rows ----
            with (
                tc.tile_pool(name="xpool", bufs=3) as xp,
                tc.tile_pool(name="pasbpool", bufs=3) as pp,
                tc.tile_pool(name="hhpool", bufs=1) as hp,
                tc.tile_pool(name="psA", bufs=3, space="PSUM") as psA,
                tc.tile_pool(name="psTa", bufs=2, space="PSUM") as psTa,
            ):
                hh_all = hp.tile([128, NB * D], FP32, tag="hh")
                # layout: [node 128, b*1024 + tg*128 + (tl*32+f)]
                for tg in range(8):
                    pa_tiles = []
                    for j, (n0, w) in enumerate(NT_SL):
                        pa = psA.tile([128, 512], FP32, tag="pa",
                                      name=f"pa{tg}_{j}")
                        pa_tiles.append(pa)
                    for tl in range(4):
                        t = 4 * tg + tl
                        xt = xp.tile([F, NLP], FP32, tag="xt", name=f"xt{t}")
                        nc.sync.dma_start(out=xt[:], in_=xT_d[t])
                        for j, (n0, w) in enumerate(NT_SL):
                            nc.tensor.matmul(
                                out=pa_tiles[j][32 * tl:32 * tl + 32, :w],
                                lhsT=gcnw[:],
                                rhs=xt[:, n0:n0 + w],
                                start=True, stop=True,
                                tile_position=(0, 32 * tl),
                            )
                    for j, (n0, w) in enumerate(NT_SL):
                        pasb = pp.tile([128, 512], FP32, tag="pasb",
                                       name=f"pasb{tg}_{j}")
                        nc.vector.tensor_copy(out=pasb[:, :w],
                                              in_=pa_tiles[j][:, :w])
                        for c4 in range(w // 128):
                            b = n0 // 128 + c4
                            ptr = psTa.tile([128, 128], FP32, tag="ptr",
                                            name=f"ptrA{tg}_{b}")
                            nc.tensor.transpose(
                                out=ptr[:],
                                in_=pasb[:, c4 * 128:(c4 + 1) * 128],
                                identity=ident[:],
                            )
                            nc.vector.tensor_copy(
                                out=hh_all[:, b * D + tg * 128:
                                           b * D + tg * 128 + 128],
                                in_=ptr[:],
                            )
                # rows -> cc_in
                for b in range(NB):
                    nc.sync.dma_start(
                        out=cc_in[b * 128:(b + 1) * 128, :],
                        in_=hh_all[:, b * D:(b + 1) * D],
                    )

            # ---- Phase B: AllGather + zero sentinel row ----
            nc.gpsimd.collective_compute(
                "AllGather",
                ALU.bypass,
                replica_groups=[list(range(NCORES))],
                ins=[cc_in[:]],
                outs=[cc_out[:VAG, :]],
            )
            nc.sync.dma_start(out=cc_out[VAG:VAG + 1, :], in_=zrow[:])

            pass_off = np.concatenate([[0], np.cumsum(pass_counts)]).astype(int)
            kmax = int(max(pass_counts))

            with tc.tile_pool(name="stpool", bufs=1) as bp:
                st = [bp.tile([128, NLP], BF16, tag=f"st{tp}", name=f"st{tp}")
                      for tp in range(16)]
                # t=31 half of the last pair never gets written by conv but is
                # read (zero-weighted) by the K=128 LSTM ih matmul
                nc.vector.memset(st[15][64:128, :], 0.0)
                with tc.tile_pool(name="h2Tpool", bufs=1) as hp2:
                    h2T = [hp2.tile([128, NLP], BF16, tag=f"h2T{q}",
                                    name=f"h2T{q}") for q in range(8)]
                    with (
                        tc.tile_pool(name="aggpool", bufs=1) as gp,
                        tc.tile_pool(name="h2spool", bufs=2) as sp2,
                        tc.tile_pool(name="psTb", bufs=2, space="PSUM") as psTb,
                    ):
                        # ---- Phase C: accumulate indirect gathers ----
                        agg_tiles = [gp.tile([128, D], FP32, tag=f"agg{b}",
                                             name=f"agg{b}") for b in range(NB)]
                        for k in range(kmax if nphases >= 3 else 1):
                            for b in range(NB):
                                if k >= pass_counts[b]:
                                    continue
                                i = int(pass_off[b]) + k
                                nc.gpsimd.indirect_dma_start(
                                    out=agg_tiles[b][:],
                                    out_offset=None,
                                    in_=cc_out[:],
                                    in_offset=bass.IndirectOffsetOnAxis(
                                        ap=idx_all[:, i:i + 1], axis=0),
                                    compute_op=(ALU.bypass if k == 0
                                                else ALU.add),
                                )

                        # ---- Phase D: scale, transpose, +bias, relu ----
                        if nphases < 4:
                            for q in range(8):
                                nc.vector.memset(h2T[q][:], 0.0)
                        for b in range(NB if nphases >= 4 else 0):
                            h2s = sp2.tile([128, D], FP32, tag="h2s",
                                           name=f"h2s{b}")
                            nc.scalar.mul(out=h2s[:], in_=agg_tiles[b][:],
                                          mul=dinv[:, b:b + 1])
                            for q in range(8):
                                ptr2 = psTb.tile([128, 128], FP32, tag="ptr",
                                                 name=f"ptrD{b}_{q}")
                                nc.tensor.transpose(
                                    out=ptr2[:],
                                    in_=h2s[:, q * 128:(q + 1) * 128],
                                    identity=ident[:],
                                )
                                nc.scalar.activation(
                                    out=h2T[q][:, b * 128:(b + 1) * 128],
                                    in_=ptr2[:],
                                    func=AF.Relu,
                                    bias=gcnb[:, :1],
                                )

                    # ---- Phase E: conv1d(k=2) as two matmuls + relu ----
                    with tc.tile_pool(name="psC", bufs=2, space="PSUM") as psC:
                        if nphases < 5:
                            for tp_z in range(16):
                                nc.vector.memset(st[tp_z][:], 0.0)
                        for t in range(TC if nphases >= 5 else 0):
                            q, tl = t // 4, t % 4
                            q2, tl2 = (t + 1) // 4, (t + 1) % 4
                            half, tp_i = t % 2, t // 2
                            for j, (n0, w) in enumerate(NT_SL):
                                pcx = psC.tile([128, 512], FP32,
                                               tag=f"pc{half}",
                                               name=f"pc{t}_{j}")
                                if tl < 3:
                                    nc.tensor.matmul(
                                        out=pcx[64 * half:64 * half + 64, :w],
                                        lhsT=cwp[:, tl * CH:(tl + 1) * CH],
                                        rhs=h2T[q][:, n0:n0 + w],
                                        start=True, stop=True,
                                        tile_position=(0, 64 * half),
                                    )
                                else:
                                    nc.tensor.matmul(
                                        out=pcx[64 * half:64 * half + 64, :w],
                                        lhsT=cwp[:, 3 * CH:4 * CH],
                                        rhs=h2T[q][:, n0:n0 + w],
                                        start=True, stop=False,
                                        tile_position=(0, 64 * half),
                                    )
                                    nc.tensor.matmul(
                                        out=pcx[64 * half:64 * half + 64, :w],
                                        lhsT=cwp[:, 4 * CH:5 * CH],
                                        rhs=h2T[q2][:, n0:n0 + w],
                                        start=False, stop=True,
                                        tile_position=(0, 64 * half),
                                    )
                                nc.scalar.activation(
                                    out=st[tp_i][64 * half:64 * half + 64,
                                                 n0:n0 + w],
                                    in_=pcx[64 * half:64 * half + 64, :w],
                                    func=AF.Relu,
                                    bias=cb[64 * half:64 * half + 64, :1],
                                )

                # ---- Phase F: LSTM over 31 steps ----
                with tc.tile_pool(name="lstmpool", bufs=2) as lp:
                  with (
                    tc.tile_pool(name="tpool", bufs=1) as tp,
                    tc.tile_pool(name="psG", bufs=2, space="PSUM") as psG,
                  ):
                    h_prev = lp.tile([128, NLP], FP32, tag="h", name="h_init")
                    c_prev = lp.tile([128, NLP], FP32, tag="c", name="c_init")
                    nc.vector.memset(h_prev[:], 0.0)
                    nc.vector.memset(c_prev[:], 0.0)
                    for t in range(TC if nphases >= 6 else 0):
                        half, tp_i = t % 2, t // 2
                        si = tp.tile([128, NLP], FP32, tag="si", name=f"si{t}")
                        sf = tp.tile([128, NLP], FP32, tag="sf", name=f"sf{t}")
                        tg_ = tp.tile([128, NLP], FP32, tag="tg", name=f"tg{t}")
                        so = tp.tile([128, NLP], FP32, tag="so", name=f"so{t}")
                        for j, (n0, w) in enumerate(NT_SL):
                            pg = [psG.tile([128, 512], FP32, tag=f"pg{g}",
                                           name=f"pg{g}_{t}_{j}")
                                  for g in range(4)]
                            for g in range(4):
                                nc.tensor.matmul(
                                    out=pg[g][:, :w],
                                    lhsT=wih[:, half * 512 + 128 * g:
                                             half * 512 + 128 * (g + 1)],
                                    rhs=st[tp_i][:, n0:n0 + w],
                                    start=True, stop=False,
                                    tile_position=(0, 0),
                                )
                                nc.tensor.matmul(
                                    out=pg[g][:, :w],
                                    lhsT=whh[:, 128 * g:128 * (g + 1)],
                                    rhs=h_prev[:, n0:n0 + w],
                                    start=False, stop=True,
                                    tile_position=(0, 0),
                                )
                            nc.scalar.activation(out=si[:, n0:n0 + w],
                                                 in_=pg[0][:, :w],
                                                 func=AF.Sigmoid,
                                                 bias=lb[:, 0:1])
                            nc.scalar.activation(out=sf[:, n0:n0 + w],
                                                 in_=pg[1][:, :w],
                                                 func=AF.Sigmoid,
                                                 bias=lb[:, 1:2])
                            nc.scalar.activation(out=tg_[:, n0:n0 + w],
                                                 in_=pg[2][:, :w],
                                                 func=AF.Tanh,
                                                 bias=lb[:, 2:3])
                            nc.scalar.activation(out=so[:, n0:n0 + w],
                                                 in_=pg[3][:, :w],
                                                 func=AF.Sigmoid,
                                                 bias=lb[:, 3:4])
                        t1 = tp.tile([128, NLP], FP32, tag="t1", name=f"t1_{t}")
                        t2 = tp.tile([128, NLP], FP32, tag="t2", name=f"t2_{t}")
                        c_new = lp.tile([128, NLP], FP32, tag="c",
                                        name=f"c{t}")
                        h_new = lp.tile([128, NLP], FP32, tag="h",
                                        name=f"h{t}")
                        tct = tp.tile([128, NLP], FP32, tag="tct",
                                      name=f"tct{t}")
                        nc.vector.tensor_tensor(out=t1[:], in0=sf[:],
                                                in1=c_prev[:], op=ALU.mult)
                        nc.vector.tensor_tensor(out=t2[:], in0=si[:],
                                                in1=tg_[:], op=ALU.mult)
                        nc.vector.tensor_tensor(out=c_new[:], in0=t1[:],
                                                in1=t2[:], op=ALU.add)
                        nc.scalar.activation(out=tct[:], in_=c_new[:],
                                             func=AF.Tanh)
                        nc.vector.tensor_tensor(out=h_new[:], in0=so[:],
                                                in1=tct[:], op=ALU.mult)
                        h_prev, c_prev = h_new, c_new

                  # ---- Phase G: FC head ----
                  if True:
                    with (
                        tc.tile_pool(name="gpool", bufs=1) as op_,
                        tc.tile_pool(name="psF", bufs=1, space="PSUM") as psF,
                    ):
                        o1 = op_.tile([64, NLP], FP32, tag="o1", name="o1")
                        for j, (n0, w) in enumerate(NT_SL):
                            p1 = psF.tile([128, 512], FP32, tag="p1",
                                          name=f"p1_{j}")
                            nc.tensor.matmul(out=p1[:64, :w], lhsT=f1w[:],
                                             rhs=h_prev[:, n0:n0 + w],
                                             start=True, stop=True,
                                             tile_position=(0, 0))
                            nc.scalar.activation(out=o1[:, n0:n0 + w],
                                                 in_=p1[:64, :w],
                                                 func=AF.Relu, bias=f1b[:, :1])
                        oT = op_.tile([128, NLP], FP32, tag="oT", name="oT")
                        for j, (n0, w) in enumerate(NT_SL):
                            p2 = psF.tile([128, 512], FP32, tag="p2",
                                          name=f"p2_{j}")
                            nc.tensor.matmul(out=p2[:, :w], lhsT=f2w[:],
                                             rhs=o1[:, n0:n0 + w],
                                             start=True, stop=True,
                                             tile_position=(0, 0))
                            nc.scalar.activation(out=oT[:, n0:n0 + w],
                                                 in_=p2[:, :w],
                                                 func=AF.Identity,
                                                 bias=f2b[:, :1])
                        nc.sync.dma_start(out=out_d[:], in_=oT[:, :NL])

    nc.compile()
    return nc


def _prep_inputs(x, edge_index, gcn_w, gcn_b, conv_w, conv_b,
                 w_ih, w_hh, b_ih, b_hh, fc1_w, fc1_b, fc2_w, fc2_b):
    x = np.asarray(x, dtype=np.float32)
    ei = np.asarray(edge_index).astype(np.int64)
    src = np.concatenate([ei[0], np.arange(N, dtype=np.int64)])
    dst = np.concatenate([ei[1], np.arange(N, dtype=np.int64)])
    deg = np.bincount(dst, minlength=N).astype(np.float32)
    dinv = 1.0 / np.sqrt(deg)  # deg >= 1 thanks to self-loops

    xs = x * dinv[:, None, None]

    # per-core gather tables
    core_data = []
    for c in range(NCORES):
        sel = (dst >= c * NL) & (dst < (c + 1) * NL)
        ld = (dst[sel] - c * NL).astype(np.int64)
        s = src[sel]
        g = (s // NL) * NLP + (s % NL)      # remap to padded AG row ids
        counts = np.bincount(ld, minlength=NL)
        kb = [int(counts[b * 128:min((b + 1) * 128, NL)].max())
              for b in range(NB)]
        core_data.append((ld, g, counts, kb))
    pass_counts = [max(core_data[c][3][b] for c in range(NCORES))
                   for b in range(NB)]
    pass_off = np.concatenate([[0], np.cumsum(pass_counts)]).astype(int)
    PASS_TOT = int(pass_off[-1])

    in_maps = []
    # shared weight prep
    gcn_w = np.ascontiguousarray(np.asarray(gcn_w, np.float32))        # [128,32]
    gcn_b = np.asarray(gcn_b, np.float32)
    gcnb_rep = np.ascontiguousarray(
        np.tile(gcn_b, 4)[:, None].astype(np.float32))                 # [128,1]
    conv_w = np.asarray(conv_w, np.float32)                            # [64,32,2]
    w0 = conv_w[:, :, 0].T          # [32, 64]
    w1 = conv_w[:, :, 1].T
    cwp = np.zeros((5, 128, CH), np.float32)
    for _k in range(3):
        cwp[_k, 32 * _k:32 * _k + 32] = w0
        cwp[_k, 32 * _k + 32:32 * _k + 64] = w1
    cwp[3, 96:128] = w0
    cwp[4, 0:32] = w1
    cwp = cwp.astype(ml_dtypes.bfloat16)
    cb_rep = np.ascontiguousarray(
        np.tile(np.asarray(conv_b, np.float32), 2)[:, None])           # [128,1]
    w_ihT0 = np.asarray(w_ih, np.float32).T                            # [64,512]
    w_ihT = np.zeros((2, 128, 4 * LH), np.float32)
    w_ihT[0, 0:64] = w_ihT0
    w_ihT[1, 64:128] = w_ihT0
    w_ihT = w_ihT.astype(ml_dtypes.bfloat16)
    w_hhT = np.ascontiguousarray(np.asarray(w_hh, np.float32).T)       # [128,512]
    lbias = (np.asarray(b_ih, np.float32) + np.asarray(b_hh, np.float32))
    lb_t = np.ascontiguousarray(lbias.reshape(4, LH).T)                # [128,4]
    fc1_w = np.ascontiguousarray(np.asarray(fc1_w, np.float32))        # [128,64]
    f1b = np.ascontiguousarray(np.asarray(fc1_b, np.float32)[:, None]) # [64,1]
    fc2_w = np.ascontiguousarray(np.asarray(fc2_w, np.float32))        # [64,128]
    f2b = np.ascontiguousarray(np.asarray(fc2_b, np.float32)[:, None]) # [128,1]

    for c in range(NCORES):
        ld, g, counts, _ = core_data[c]
        order = np.argsort(ld, kind="stable")
        lds, gs = ld[order], g[order]
        gidx = np.full((PASS_TOT, 128), SENT, dtype=np.int32)
        # position of each edge within its node's list
        starts = np.concatenate([[0], np.cumsum(counts)]).astype(np.int64)
        kpos = np.arange(len(lds)) - starts[lds]
        blk = lds // 128
        lane = lds % 128
        rows = pass_off[blk] + kpos
        gidx[rows, lane] = gs
        gidxT = np.ascontiguousarray(gidx.T)                           # [128,PASS_TOT]

        xT = np.zeros((T, F, NLP), dtype=np.float32)
        xT[:, :, :NL] = xs[c * NL:(c + 1) * NL].transpose(1, 2, 0)

        dv = np.ones(NLP, dtype=np.float32)
        dv[:NL] = dinv[c * NL:(c + 1) * NL]
        dv_t = np.ascontiguousarray(dv.reshape(NB, 128).T)             # [128,NB]

        in_maps.append({
            "xT": xT,
            "gidx": gidxT,
            "dinv": dv_t,
            "gcn_w": gcn_w,
            "gcn_b_rep": gcnb_rep,
            "conv_wp": cwp,
            "conv_b_rep": cb_rep,
            "w_ihT": w_ihT,
            "w_hhT": w_hhT,
            "lstm_b": lb_t,
            "fc1_w": fc1_w,
            "fc1_b": f1b,
            "fc2_w": fc2_w,
            "fc2_b": f2b,
        })
    return pass_counts, in_maps


_RESULTS_CACHE = {}


def _bench_pjrt(nc, in_maps, n_iters=8):
    """Time the compiled NEFF executable on the 8 cores via PJRT.

    Returns (results_list, best_wall_seconds). Mirrors
    bass2jax.run_bass_via_pjrt's multi-core path but keeps the executable so
    we can re-run it with staged device inputs.
    """
    import time
    import jax
    import numpy as np
    from jax.sharding import Mesh, PartitionSpec
    from jax.experimental.shard_map import shard_map
    import concourse.mybir as mybir
    from concourse.bass2jax import _bass_exec_p, install_neuronx_cc_hook, partition_id_tensor

    install_neuronx_cc_hook()
    partition_name = nc.partition_id_tensor.name if nc.partition_id_tensor else None
    in_names, out_names, out_avals, zero_outs = [], [], [], []
    for alloc in nc.m.functions[0].allocations:
        if not isinstance(alloc, mybir.MemoryLocationSet):
            continue
        name = alloc.memorylocations[0].name
        if alloc.kind == "ExternalInput":
            if name != partition_name:
                in_names.append(name)
        elif alloc.kind == "ExternalOutput":
            shape = tuple(alloc.tensor_shape)
            dtype = mybir.dt.np(alloc.dtype)
            out_names.append(name)
            out_avals.append(jax.core.ShapedArray(shape, dtype))
            zero_outs.append(np.zeros(shape, dtype))
    n_params = len(in_names)
    all_in_names = list(in_names) + list(out_names)
    if partition_name is not None:
        all_in_names.append(partition_name)

    def _body(*args):
        operands = list(args)
        if partition_name is not None:
            operands.append(partition_id_tensor())
        outs = _bass_exec_p.bind(
            *operands,
            out_avals=tuple(out_avals),
            in_names=tuple(all_in_names),
            out_names=tuple(out_names),
            lowering_input_output_aliases=(),
            sim_require_finite=True,
            sim_require_nnan=True,
            nc=nc,
        )
        return tuple(outs)

    n_cores = len(in_maps)
    devices = jax.devices()[:n_cores]
    mesh = Mesh(np.asarray(devices), ("core",))
    in_specs = (PartitionSpec("core"),) * (n_params + len(out_names))
    out_specs = (PartitionSpec("core"),) * len(out_names)
    sharded = jax.jit(
        shard_map(_body, mesh=mesh, in_specs=in_specs, out_specs=out_specs,
                  check_rep=False),
        keep_unused=True,
    )
    sharding = jax.sharding.NamedSharding(mesh, PartitionSpec("core"))
    concat_in = [
        jax.device_put(
            np.concatenate([np.asarray(in_maps[c][nm]) for c in range(n_cores)], axis=0),
            sharding)
        for nm in in_names
    ]
    concat_zeros = [
        jax.device_put(np.zeros((n_cores * z.shape[0], *z.shape[1:]), z.dtype),
                       sharding)
        for z in zero_outs
    ]
    out_arrs = sharded(*concat_in, *concat_zeros)
    jax.block_until_ready(out_arrs)
    best = float("inf")
    for _ in range(n_iters):
        t0 = time.perf_counter()
        out_arrs2 = sharded(*concat_in, *concat_zeros)
        jax.block_until_ready(out_arrs2)
        best = min(best, time.perf_counter() - t0)
    results = [
        {nm: np.asarray(out_arrs[i]).reshape(n_cores, *out_avals[i].shape)[c]
         for i, nm in enumerate(out_names)}
        for c in range(n_cores)
    ]
    return results, best


def run(trace=False, bench=False, **inputs):
    pass_counts, in_maps = _prep_inputs(**inputs)
    nc = _build_program(pass_counts)
    if bench:
        results, best = _bench_pjrt(nc, in_maps)
        _RESULTS_CACHE["best_wall_s"] = best
    else:
        res = run_bass_kernel_spmd(nc, in_maps, core_ids=list(range(NCORES)),
                                   trace=trace)
        results = res.results
        _RESULTS_CACHE["last"] = res
    out = np.empty((N, F), dtype=np.float32)
    for c in range(NCORES):
        out[c * NL:(c + 1) * NL, :] = results[c]["outT"].T
    return out


def kernel(**inputs) -> np.ndarray:
    return run(trace=False, **inputs)


# revision 10
# speedup vs baseline: 1.3244x; 1.3244x over previous
"""CNN-LSTM-GCN kernel for 8 Trainium2 NeuronCores.

Self-contained: hardcodes shapes from the problem spec.
  x:          [10000, 32, 128] f32
  edge_index: [2, 160000] int
  output:     [10000, 128] f32

Sharding: nodes split 1250/core (padded to 1280), edges partitioned by dst,
weights replicated. One AllGather for the GCN source rows; the graph
aggregation is done with indirect-DMA gathers that accumulate (CCE add).
"""

import numpy as np
import ml_dtypes

import concourse.bass as bass
import concourse.bacc as bacc
import concourse.mybir as mybir
import concourse.tile as tile
from concourse.bass_utils import run_bass_kernel_spmd
from concourse.masks import make_identity

# ---- problem constants ----
N = 10000
T = 32
F = 128
GH = 32          # GCN hidden
CH = 64          # conv channels
LH = 128         # LSTM hidden
NCORES = 8
NL = N // NCORES          # 1250 real nodes per core
NLP = 1280                # padded nodes per core
NB = NLP // 128           # 10 blocks of 128 dst nodes
VAG = NCORES * NLP        # 10240 rows in the all-gathered table
SENT = VAG                # sentinel index -> zero row
D = T * GH                # 1024 features per node row after GCN transform
NT_SL = [(0, 512), (512, 512), (1024, 256)]  # node tiles
TC = T - 1                # conv output timesteps (31)

FP32 = mybir.dt.float32
BF16 = mybir.dt.bfloat16
INT32 = mybir.dt.int32
AF = mybir.ActivationFunctionType
ALU = mybir.AluOpType


def _build_program(pass_counts, nphases=99):
    """Trace the SPMD bass program. pass_counts[b] = gather passes for block b
    (shared by all cores). nphases limits how many phases run (debug)."""
    nc = bacc.Bacc("TRN2", target_bir_lowering=False, debug=False,
                   num_devices=NCORES)

    PASS_TOT = int(sum(pass_counts))

    # ---- I/O ----
    xT_d = nc.dram_tensor("xT", [T, F, NLP], FP32, kind="ExternalInput")
    gidx_d = nc.dram_tensor("gidx", [128, PASS_TOT], INT32, kind="ExternalInput")
    dinv_d = nc.dram_tensor("dinv", [128, NB], FP32, kind="ExternalInput")
    gcnw_d = nc.dram_tensor("gcn_w", [F, GH], FP32, kind="ExternalInput")
    gcnb_d = nc.dram_tensor("gcn_b_rep", [128, 1], FP32, kind="ExternalInput")
    cwp_d = nc.dram_tensor("conv_wp", [5, 128, CH], BF16, kind="ExternalInput")
    cb_d = nc.dram_tensor("conv_b_rep", [128, 1], FP32, kind="ExternalInput")
    wih_d = nc.dram_tensor("w_ihT", [2, 128, 4 * LH], BF16, kind="ExternalInput")
    whh_d = nc.dram_tensor("w_hhT", [LH, 4 * LH], FP32, kind="ExternalInput")
    lb_d = nc.dram_tensor("lstm_b", [128, 4], FP32, kind="ExternalInput")
    f1w_d = nc.dram_tensor("fc1_w", [LH, 64], FP32, kind="ExternalInput")
    f1b_d = nc.dram_tensor("fc1_b", [64, 1], FP32, kind="ExternalInput")
    f2w_d = nc.dram_tensor("fc2_w", [64, F], FP32, kind="ExternalInput")
    f2b_d = nc.dram_tensor("fc2_b", [128, 1], FP32, kind="ExternalInput")
    out_d = nc.dram_tensor("outT", [F, NL], FP32, kind="ExternalOutput")

    cc_in = nc.dram_tensor("cc_in", [NLP, D], FP32)
    cc_out = nc.dram_tensor("cc_out", [VAG + 1, D], FP32, addr_space="Shared")

    with tile.TileContext(nc) as tc:
        with tc.tile_pool(name="wpool", bufs=1) as wp:
            # ---- persistent weights ----
            ident = wp.tile([128, 128], FP32)
            make_identity(nc, ident[:])
            gcnw = wp.tile([F, GH], FP32)
            nc.sync.dma_start(out=gcnw[:], in_=gcnw_d[:])
            gcnb = wp.tile([128, 1], FP32)
            nc.sync.dma_start(out=gcnb[:], in_=gcnb_d[:])
            cwp = wp.tile([128, 5 * CH], BF16)
            for k5 in range(5):
                nc.sync.dma_start(out=cwp[:, k5 * CH:(k5 + 1) * CH],
                                  in_=cwp_d[k5])
            cb = wp.tile([128, 1], FP32)
            nc.sync.dma_start(out=cb[:], in_=cb_d[:])
            wih = wp.tile([128, 2 * 4 * LH], BF16)
            for k2 in range(2):
                nc.sync.dma_start(out=wih[:, k2 * 4 * LH:(k2 + 1) * 4 * LH],
                                  in_=wih_d[k2])
            whh = wp.tile([LH, 4 * LH], FP32)
            nc.sync.dma_start(out=whh[:], in_=whh_d[:])
            lb = wp.tile([128, 4], FP32)
            nc.sync.dma_start(out=lb[:], in_=lb_d[:])
            f1w = wp.tile([LH, 64], FP32)
            nc.sync.dma_start(out=f1w[:], in_=f1w_d[:])
            f1b = wp.tile([64, 1], FP32)
            nc.sync.dma_start(out=f1b[:], in_=f1b_d[:])
            f2w = wp.tile([64, F], FP32)
            nc.sync.dma_start(out=f2w[:], in_=f2w_d[:])
            f2b = wp.tile([128, 1], FP32)
            nc.sync.dma_start(out=f2b[:], in_=f2b_d[:])
            idx_all = wp.tile([128, PASS_TOT], INT32)
            nc.sync.dma_start(out=idx_all[:], in_=gidx_d[:])
            dinv = wp.tile([128, NB], FP32)
            nc.sync.dma_start(out=dinv[:], in_=dinv_d[:])
            zrow = wp.tile([1, D], FP32)
            nc.vector.memset(zrow[:], 0.0)

            # ---- Phase A: h^T = gcn_w^T @ x^T (scaled x), transpose to rows ----
            with (
                tc.tile_pool(name="xpool", bufs=3) as xp,
                tc.tile_pool(name="pasbpool", bufs=3) as pp,
                tc.tile_pool(name="hhpool", bufs=1) as hp,
                tc.tile_pool(name="psA", bufs=3, space="PSUM") as psA,
                tc.tile_pool(name="psTa", bufs=2, space="PSUM") as psTa,
            ):
                hh_all = hp.tile([128, NB * D], FP32, tag="hh")
                # layout: [node 128, b*1024 + tg*128 + (tl*32+f)]
                for tg in range(8):
                    pa_tiles = []
                    for j, (n0, w) in enumerate(NT_SL):
                        pa = psA.tile([128, 512], FP32, tag="pa",
                                      name=f"pa{tg}_{j}")
                        pa_tiles.append(pa)
                    for tl in range(4):
                        t = 4 * tg + tl
                        xt = xp.tile([F, NLP], FP32, tag="xt", name=f"xt{t}")
                        nc.sync.dma_start(out=xt[:], in_=xT_d[t])
                        for j, (n0, w) in enumerate(NT_SL):
                            nc.tensor.matmul(
                                out=pa_tiles[j][32 * tl:32 * tl + 32, :w],
                                lhsT=gcnw[:],
                                rhs=xt[:, n0:n0 + w],
                                start=True, stop=True,
                                tile_position=(0, 32 * tl),
                            )
                    for j, (n0, w) in enumerate(NT_SL):
                        pasb = pp.tile([128, 512], FP32, tag="pasb",
                                       name=f"pasb{tg}_{j}")
                        nc.vector.tensor_copy(out=pasb[:, :w],
                                              in_=pa_tiles[j][:, :w])
                        for c4 in range(w // 128):
                            b = n0 // 128 + c4
                            ptr = psTa.tile([128, 128], FP32, tag="ptr",
                                            name=f"ptrA{tg}_{b}")
                            nc.tensor.transpose(
                                out=ptr[:],
                                in_=pasb[:, c4 * 128:(c4 + 1) * 128],
                                identity=ident[:],
                            )
                            nc.vector.tensor_copy(
                                out=hh_all[:, b * D + tg * 128:
                                           b * D + tg * 128 + 128],
                                in_=ptr[:],
                            )
                # rows -> cc_in
                for b in range(NB):
                    nc.sync.dma_start(
                        out=cc_in[b * 128:(b + 1) * 128, :],
                        in_=hh_all[:, b * D:(b + 1) * D],
                    )

            # ---- Phase B: AllGather + zero sentinel row ----
            nc.gpsimd.collective_compute(
                "AllGather",
                ALU.bypass,
                replica_groups=[list(range(NCORES))],
                ins=[cc_in[:]],
                outs=[cc_out[:VAG, :]],
            )
            nc.sync.dma_start(out=cc_out[VAG:VAG + 1, :], in_=zrow[:])

            pass_off = np.concatenate([[0], np.cumsum(pass_counts)]).astype(int)
            kmax = int(max(pass_counts))

            with tc.tile_pool(name="stpool", bufs=1) as bp:
                st = [bp.tile([128, NLP], BF16, tag=f"st{tp}", name=f"st{tp}")
                      for tp in range(16)]
                # t=31 half of the last pair never gets written by conv but is
                # read (zero-weighted) by the K=128 LSTM ih matmul
                nc.vector.memset(st[15][64:128, :], 0.0)
                with tc.tile_pool(name="h2Tpool", bufs=1) as hp2:
                    h2T = [hp2.tile([128, NLP], BF16, tag=f"h2T{q}",
                                    name=f"h2T{q}") for q in range(8)]
                    with (
                        tc.tile_pool(name="aggpool", bufs=1) as gp,
                        tc.tile_pool(name="h2spool", bufs=2) as sp2,
                        tc.tile_pool(name="psTb", bufs=2, space="PSUM") as psTb,
                    ):
                        # ---- Phase C: accumulate indirect gathers ----
                        agg_tiles = [gp.tile([128, D], FP32, tag=f"agg{b}",
                                             name=f"agg{b}") for b in range(NB)]
                        for k in range(kmax if nphases >= 3 else 1):
                            for b in range(NB):
                                if k >= pass_counts[b]:
                                    continue
                                i = int(pass_off[b]) + k
                                nc.gpsimd.indirect_dma_start(
                                    out=agg_tiles[b][:],
                                    out_offset=None,
                                    in_=cc_out[:],
                                    in_offset=bass.IndirectOffsetOnAxis(
                                        ap=idx_all[:, i:i + 1], axis=0),
                                    compute_op=(ALU.bypass if k == 0
                                                else ALU.add),
                                )

                        # ---- Phase D: scale, transpose, +bias, relu ----
                        if nphases < 4:
                            for q in range(8):
                                nc.vector.memset(h2T[q][:], 0.0)
                        for b in range(NB if nphases >= 4 else 0):
                            h2s = sp2.tile([128, D], FP32, tag="h2s",
                                           name=f"h2s{b}")
                            nc.scalar.mul(out=h2s[:], in_=agg_tiles[b][:],
                                          mul=dinv[:, b:b + 1])
                            for q in range(8):
                                ptr2 = psTb.tile([128, 128], FP32, tag="ptr",
                                                 name=f"ptrD{b}_{q}")
                                nc.tensor.transpose(
                                    out=ptr2[:],
                                    in_=h2s[:, q * 128:(q + 1) * 128],
                                    identity=ident[:],
                                )
                                nc.scalar.activation(
                                    out=h2T[q][:, b * 128:(b + 1) * 128],
                                    in_=ptr2[:],
                                    func=AF.Relu,
                                    bias=gcnb[:, :1],
                                )

                    # ---- Phase E: conv1d(k=2) as two matmuls + relu ----
                    with tc.tile_pool(name="psC", bufs=2, space="PSUM") as psC:
                        if nphases < 5:
                            for tp_z in range(16):
                                nc.vector.memset(st[tp_z][:], 0.0)
                        for t in range(TC if nphases >= 5 else 0):
                            q, tl = t // 4, t % 4
                            q2, tl2 = (t + 1) // 4, (t + 1) % 4
                            half, tp_i = t % 2, t // 2
                            for j, (n0, w) in enumerate(NT_SL):
                                pcx = psC.tile([128, 512], FP32,
                                               tag=f"pc{half}",
                                               name=f"pc{t}_{j}")
                                if tl < 3:
                                    nc.tensor.matmul(
                                        out=pcx[64 * half:64 * half + 64, :w],
                                        lhsT=cwp[:, tl * CH:(tl + 1) * CH],
                                        rhs=h2T[q][:, n0:n0 + w],
                                        start=True, stop=True,
                                        tile_position=(0, 64 * half),
                                    )
                                else:
                                    nc.tensor.matmul(
                                        out=pcx[64 * half:64 * half + 64, :w],
                                        lhsT=cwp[:, 3 * CH:4 * CH],
                                        rhs=h2T[q][:, n0:n0 + w],
                                        start=True, stop=False,
                                        tile_position=(0, 64 * half),
                                    )
                                    nc.tensor.matmul(
                                        out=pcx[64 * half:64 * half + 64, :w],
                                        lhsT=cwp[:, 4 * CH:5 * CH],
                                        rhs=h2T[q2][:, n0:n0 + w],
                                        start=False, stop=True,
                                        tile_position=(0, 64 * half),
                                    )
                                nc.scalar.activation(
                                    out=st[tp_i][64 * half:64 * half + 64,
                                                 n0:n0 + w],
                                    in_=pcx[64 * half:64 * half + 64, :w],
                                    func=AF.Relu,
                                    bias=cb[64 * half:64 * half + 64, :1],
                                )

                # ---- Phase F: LSTM over 31 steps ----
                with tc.tile_pool(name="lstmpool", bufs=2) as lp:
                  with (
                    tc.tile_pool(name="tpool", bufs=1) as tp,
                    tc.tile_pool(name="psG", bufs=2, space="PSUM") as psG,
                  ):
                    h_prev = lp.tile([128, NLP], FP32, tag="h", name="h_init")
                    c_prev = lp.tile([128, NLP], FP32, tag="c", name="c_init")
                    nc.vector.memset(h_prev[:], 0.0)
                    nc.vector.memset(c_prev[:], 0.0)
                    for t in range(TC if nphases >= 6 else 0):
                        half, tp_i = t % 2, t // 2
                        si = tp.tile([128, NLP], FP32, tag="si", name=f"si{t}")
                        sf = tp.tile([128, NLP], FP32, tag="sf", name=f"sf{t}")
                        tg_ = tp.tile([128, NLP], FP32, tag="tg", name=f"tg{t}")
                        so = tp.tile([128, NLP], FP32, tag="so", name=f"so{t}")
                        for j, (n0, w) in enumerate(NT_SL):
                            pg = [psG.tile([128, 512], FP32, tag=f"pg{g}",
                                           name=f"pg{g}_{t}_{j}")
                                  for g in range(4)]
                            for g in range(4):
                                nc.tensor.matmul(
                                    out=pg[g][:, :w],
                                    lhsT=wih[:, half * 512 + 128 * g:
                                             half * 512 + 128 * (g + 1)],
                                    rhs=st[tp_i][:, n0:n0 + w],
                                    start=True, stop=False,
                                    tile_position=(0, 0),
                                )
                                nc.tensor.matmul(
                                    out=pg[g][:, :w],
                                    lhsT=whh[:, 128 * g:128 * (g + 1)],
                                    rhs=h_prev[:, n0:n0 + w],
                                    start=False, stop=True,
                                    tile_position=(0, 0),
                                )
                            nc.scalar.activation(out=si[:, n0:n0 + w],
                                                 in_=pg[0][:, :w],
                                                 func=AF.Sigmoid,
                                                 bias=lb[:, 0:1])
                            nc.scalar.activation(out=sf[:, n0:n0 + w],
                                                 in_=pg[1][:, :w],
                                                 func=AF.Sigmoid,
                                                 bias=lb[:, 1:2])
                            nc.scalar.activation(out=tg_[:, n0:n0 + w],
                                                 in_=pg[2][:, :w],
                                                 func=AF.Tanh,
                                                 bias=lb[:, 2:3])
                            nc.scalar.activation(out=so[:, n0:n0 + w],
                                                 in_=pg[3][:, :w],
                                                 func=AF.Sigmoid,
                                                 bias=lb[:, 3:4])
                        t1 = tp.tile([128, NLP], FP32, tag="t1", name=f"t1_{t}")
                        t2 = tp.tile([128, NLP], FP32, tag="t2", name=f"t2_{t}")
                        c_new = lp.tile([128, NLP], FP32, tag="c",
                                        name=f"c{t}")
                        h_new = lp.tile([128, NLP], FP32, tag="h",
                                        name=f"h{t}")
                        tct = tp.tile([128, NLP], FP32, tag="tct",
                                      name=f"tct{t}")
                        nc.vector.tensor_tensor(out=t1[:], in0=sf[:],
                                                in1=c_prev[:], op=ALU.mult)
                        nc.vector.tensor_tensor(out=t2[:], in0=si[:],
                                                in1=tg_[:], op=ALU.mult)
                        nc.vector.tensor_tensor(out=c_new[:], in0=t1[:],
                                                in1=t2[:], op=ALU.add)
                        nc.scalar.activation(out=tct[:], in_=c_new[:],
                                             func=AF.Tanh)
                        nc.vector.tensor_tensor(out=h_new[:], in0=so[:],
                                                in1=tct[:], op=ALU.mult)
                        h_prev, c_prev = h_new, c_new

                  # ---- Phase G: FC head ----
                  if True:
                    with (
                        tc.tile_pool(name="gpool", bufs=1) as op_,
                        tc.tile_pool(name="psF", bufs=1, space="PSUM") as psF,
                    ):
                        o1 = op_.tile([64, NLP], FP32, tag="o1", name="o1")
                        for j, (n0, w) in enumerate(NT_SL):
                            p1 = psF.tile([128, 512], FP32, tag="p1",
                                          name=f"p1_{j}")
                            nc.tensor.matmul(out=p1[:64, :w], lhsT=f1w[:],
                                             rhs=h_prev[:, n0:n0 + w],
                                             start=True, stop=True,
                                             tile_position=(0, 0))
                            nc.scalar.activation(out=o1[:, n0:n0 + w],
                                                 in_=p1[:64, :w],
                                                 func=AF.Relu, bias=f1b[:, :1])
                        oT = op_.tile([128, NLP], FP32, tag="oT", name="oT")
                        for j, (n0, w) in enumerate(NT_SL):
                            p2 = psF.tile([128, 512], FP32, tag="p2",
                                          name=f"p2_{j}")
                            nc.tensor.matmul(out=p2[:, :w], lhsT=f2w[:],
                                             rhs=o1[:, n0:n0 + w],
                                             start=True, stop=True,
                                             tile_position=(0, 0))
                            nc.scalar.activation(out=oT[:, n0:n0 + w],
                                                 in_=p2[:, :w],
                                                 func=AF.Identity,
                                                 bias=f2b[:, :1])
                        nc.sync.dma_start(out=out_d[:], in_=oT[:, :NL])

    nc.compile()
    return nc


def _prep_inputs(x, edge_index, gcn_w, gcn_b, conv_w, conv_b,
                 w_ih, w_hh, b_ih, b_hh, fc1_w, fc1_b, fc2_w, fc2_b):
    x = np.asarray(x, dtype=np.float32)
    ei = np.asarray(edge_index).astype(np.int64)
    src = np.concatenate([ei[0], np.arange(N, dtype=np.int64)])
    dst = np.concatenate([ei[1], np.arange(N, dtype=np.int64)])
    deg = np.bincount(dst, minlength=N).astype(np.float32)
    dinv = 1.0 / np.sqrt(deg)  # deg >= 1 thanks to self-loops

    xs = x * dinv[:, None, None]

    # per-core gather tables
    core_data = []
    for c in range(NCORES):
        sel = (dst >= c * NL) & (dst < (c + 1) * NL)
        ld = (dst[sel] - c * NL).astype(np.int64)
        s = src[sel]
        g = (s // NL) * NLP + (s % NL)      # remap to padded AG row ids
        counts = np.bincount(ld, minlength=NL)
        kb = [int(counts[b * 128:min((b + 1) * 128, NL)].max())
              for b in range(NB)]
        core_data.append((ld, g, counts, kb))
    pass_counts = [max(core_data[c][3][b] for c in range(NCORES))
                   for b in range(NB)]
    pass_off = np.concatenate([[0], np.cumsum(pass_counts)]).astype(int)
    PASS_TOT = int(pass_off[-1])

    in_maps = []
    # shared weight prep
    gcn_w = np.ascontiguousarray(np.asarray(gcn_w, np.float32))        # [128,32]
    gcn_b = np.asarray(gcn_b, np.float32)
    gcnb_rep = np.ascontiguousarray(
        np.tile(gcn_b, 4)[:, None].astype(np.float32))                 # [128,1]
    conv_w = np.asarray(conv_w, np.float32)                            # [64,32,2]
    w0 = conv_w[:, :, 0].T          # [32, 64]
    w1 = conv_w[:, :, 1].T
    cwp = np.zeros((5, 128, CH), np.float32)
    for _k in range(3):
        cwp[_k, 32 * _k:32 * _k + 32] = w0
        cwp[_k, 32 * _k + 32:32 * _k + 64] = w1
    cwp[3, 96:128] = w0
    cwp[4, 0:32] = w1
    cwp = cwp.astype(ml_dtypes.bfloat16)
    cb_rep = np.ascontiguousarray(
        np.tile(np.asarray(conv_b, np.float32), 2)[:, None])           # [128,1]
    w_ihT0 = np.asarray(w_ih, np.float32).T                            # [64,512]
    w_ihT = np.zeros((2, 128, 4 * LH), np.float32)
    w_ihT[0, 0:64] = w_ihT0
    w_ihT[1, 64:128] = w_ihT0
    w_ihT = w_ihT.astype(ml_dtypes.bfloat16)
    w_hhT = np.ascontiguousarray(np.asarray(w_hh, np.float32).T)       # [128,512]
    lbias = (np.asarray(b_ih, np.float32) + np.asarray(b_hh, np.float32))
    lb_t = np.ascontiguousarray(lbias.reshape(4, LH).T)                # [128,4]
    fc1_w = np.ascontiguousarray(np.asarray(fc1_w, np.float32))        # [128,64]
    f1b = np.ascontiguousarray(np.asarray(fc1_b, np.float32)[:, None]) # [64,1]
    fc2_w = np.ascontiguousarray(np.asarray(fc2_w, np.float32))        # [64,128]
    f2b = np.ascontiguousarray(np.asarray(fc2_b, np.float32)[:, None]) # [128,1]

    for c in range(NCORES):
        ld, g, counts, _ = core_data[c]
        order = np.argsort(ld, kind="stable")
        lds, gs = ld[order], g[order]
        gidx = np.full((PASS_TOT, 128), SENT, dtype=np.int32)
        # position of each edge within its node's list
        starts = np.concatenate([[0], np.cumsum(counts)]).astype(np.int64)
        kpos = np.arange(len(lds)) - starts[lds]
        blk = lds // 128
        lane = lds % 128
        rows = pass_off[blk] + kpos
        gidx[rows, lane] = gs
        gidxT = np.ascontiguousarray(gidx.T)                           # [128,PASS_TOT]

        xT = np.zeros((T, F, NLP), dtype=np.float32)
        xT[:, :, :NL] = xs[c * NL:(c + 1) * NL].transpose(1, 2, 0)

        dv = np.ones(NLP, dtype=np.float32)
        dv[:NL] = dinv[c * NL:(c + 1) * NL]
        dv_t = np.ascontiguousarray(dv.reshape(NB, 128).T)             # [128,NB]

        in_maps.append({
            "xT": xT,
            "gidx": gidxT,
            "dinv": dv_t,
            "gcn_w": gcn_w,
            "gcn_b_rep": gcnb_rep,
            "conv_wp": cwp,
            "conv_b_rep": cb_rep,
            "w_ihT": w_ihT,
            "w_hhT": w_hhT,
            "lstm_b": lb_t,
            "fc1_w": fc1_w,
            "fc1_b": f1b,
            "fc2_w": fc2_w,
            "fc2_b": f2b,
        })
    return pass_counts, in_maps


_RESULTS_CACHE = {}


def _bench_pjrt(nc, in_maps, n_iters=8):
    """Time the compiled NEFF executable on the 8 cores via PJRT.

    Returns (results_list, best_wall_seconds). Mirrors
    bass2jax.run_bass_via_pjrt's multi-core path but keeps the executable so
    we can re-run it with staged device inputs.
    """
    import time
    import jax
    import numpy as np
    from jax.sharding import Mesh, PartitionSpec
    from jax.experimental.shard_map import shard_map
    import concourse.mybir as mybir
    from concourse.bass2jax import _bass_exec_p, install_neuronx_cc_hook, partition_id_tensor

    install_neuronx_cc_hook()
    partition_name = nc.partition_id_tensor.name if nc.partition_id_tensor else None
    in_names, out_names, out_avals, zero_outs = [], [], [], []
    for alloc in nc.m.functions[0].allocations:
        if not isinstance(alloc, mybir.MemoryLocationSet):
            continue
        name = alloc.memorylocations[0].name
        if alloc.kind == "ExternalInput":
            if name != partition_name:
                in_names.append(name)
        elif alloc.kind == "ExternalOutput":
            shape = tuple(alloc.tensor_shape)
            dtype = mybir.dt.np(alloc.dtype)
            out_names.append(name)
            out_avals.append(jax.core.ShapedArray(shape, dtype))
            zero_outs.append(np.zeros(shape, dtype))
    n_params = len(in_names)
    all_in_names = list(in_names) + list(out_names)
    if partition_name is not None:
        all_in_names.append(partition_name)

    def _body(*args):
        operands = list(args)
        if partition_name is not None:
            operands.append(partition_id_tensor())
        outs = _bass_exec_p.bind(
            *operands,
            out_avals=tuple(out_avals),
            in_names=tuple(all_in_names),
            out_names=tuple(out_names),
            lowering_input_output_aliases=(),
            sim_require_finite=True,
            sim_require_nnan=True,
            nc=nc,
        )
        return tuple(outs)

    n_cores = len(in_maps)
    devices = jax.devices()[:n_cores]
    mesh = Mesh(np.asarray(devices), ("core",))
    in_specs = (PartitionSpec("core"),) * (n_params + len(out_names))
    out_specs = (PartitionSpec("core"),) * len(out_names)
    sharded = jax.jit(
        shard_map(_body, mesh=mesh, in_specs=in_specs, out_specs=out_specs,
                  check_rep=False),
        keep_unused=True,
    )
    sharding = jax.sharding.NamedSharding(mesh, PartitionSpec("core"))
    concat_in = [
        jax.device_put(
            np.concatenate([np.asarray(in_maps[c][nm]) for c in range(n_cores)], axis=0),
            sharding)
        for nm in in_names
    ]
    concat_zeros = [
        jax.device_put(np.zeros((n_cores * z.shape[0], *z.shape[1:]), z.dtype),
                       sharding)
        for z in zero_outs
    ]
    out_arrs = sharded(*concat_in, *concat_zeros)
    jax.block_until_ready(out_arrs)
    best = float("inf")
    for _ in range(n_iters):
        t0 = time.perf_counter()
        out_arrs2 = sharded(*concat_in, *concat_zeros)
        jax.block_until_ready(out_arrs2)
        best = min(best, time.perf_counter() - t0)
    # pipelined launches: amortize the axon dispatch round-trip
    PIPE = 16
    for _ in range(2):
        t0 = time.perf_counter()
        outs = [sharded(*concat_in, *concat_zeros) for _ in range(PIPE)]
        jax.block_until_ready(outs)
        dt = (time.perf_counter() - t0) / PIPE
        best_piped = dt
    _RESULTS_CACHE["piped_wall_s"] = best_piped
    results = [
        {nm: np.asarray(out_arrs[i]).reshape(n_cores, *out_avals[i].shape)[c]
         for i, nm in enumerate(out_names)}
        for c in range(n_cores)
    ]
    return results, best


def run(trace=False, bench=False, **inputs):
    pass_counts, in_maps = _prep_inputs(**inputs)
    nc = _build_program(pass_counts)
    if bench:
        results, best = _bench_pjrt(nc, in_maps)
        _RESULTS_CACHE["best_wall_s"] = best
    else:
        res = run_bass_kernel_spmd(nc, in_maps, core_ids=list(range(NCORES)),
                                   trace=trace)
        results = res.results
        _RESULTS_CACHE["last"] = res
    out = np.empty((N, F), dtype=np.float32)
    for c in range(NCORES):
        out[c * NL:(c + 1) * NL, :] = results[c]["outT"].T
    return out


def kernel(**inputs) -> np.ndarray:
    return run(trace=False, **inputs)
